# revision 24
# baseline (speedup 1.0000x reference)
"""AdaptiveKernelFC Trainium2 kernel (8-core data parallel), v3.

Math: the reference builds per-sample filters w[n,p,c,kh,kw] =
x[n,c,kh,kw]*Wk[p] + bk[p] and convolves x[n] with them (7x7 kernel ==
feature map size, pad 3).  The conv factors exactly:

    y[n,p,i,j] = Wk[p]*S1[n,i,j] + bk[p]*S2[n,i,j] + b_adap[p]

with S1 the 2D autocorrelation of x[n] (summed over channels) and S2
the 7x7 box-filter correlation of the channel sums.  Both come from one
fused matmul pair per sample with stationary [x | ones]: PSUM rows 0:49
hold the spatial Gram matrix G[r, q], rows 49:98 hold the channel sums
xs[q] replicated.

The diagonal band T[p, (n, t)] = row p's value at column (r-24)+t
(r = p mod 49) is produced by a DRAM staging round trip (SBUF DMAs
cannot mix partition and element steps; DRAM is flat): dump rows at
stride 192 starting at column 24, then ONE gather per (region, half)
with row stride 193 -- the +1 slope realizes the per-row shift.  The
staging tensors are inline zero constants, so the out-of-band positions
the gather sweeps through are always finite zeros/neighbor data; a
single mask-multiply (f32 mask broadcast over samples, bf16 output)
zeroes the wrapped positions and casts for the TensorEngine in one op.

The final stage contracts T directly against Q (98, 256) where
Q[p, :] = Wk if p < 49 else bk (built once off the critical path by a
tiny selector matmul), so the selector reduction, bias staging and R
assembly of earlier versions all disappear:

    y[ck*128+p, (n,s)] = (Q[:, ck]^T Tbf)[p, (n,s)] + b_adap
    (b_adap added by the PSUM->SBUF tensor_scalar_add move).

Everything is split into two sample-halves so the second half's Gram
matmuls and round trip overlap the first half's output pipeline.

Sharding: pure data parallel, batch N=32 split 4 samples/core across 8
cores; params replicated; outputs concatenated.
"""

import os
import numpy as np

import concourse.bass as bass
import concourse.bacc as bacc
import concourse.mybir as mybir
import concourse.tile as tile
from concourse.tile_rust import add_dep_helper
from concourse.ap import AP
from concourse.bass_utils import run_bass_kernel_spmd

N, C, H, W = 32, 256, 7, 7
P = 256
NCORES = 8
B = N // NCORES          # samples per core
HW = H * W               # 49
ROWS = 2 * HW            # 98 PSUM rows (G region then xs region)
FREE = B * HW            # 196 free columns (n, t)
HF = 2 * HW              # 98 free columns per half
SROW = 224               # staging row stride (bf16 elems)
SSEP = 112               # per-sample block separation within a row
F32 = mybir.dt.float32
BF16 = mybir.dt.bfloat16

_cached = {}
last_exec_time_ns = None


def _mask_np():
    # Pre-shear mask on G columns: M[p, q] = 1 iff band entry
    # (p, t = q - r + 24) is a valid (no-wrap) 2D lag, r = p mod 49.
    m = np.zeros((ROWS, HW), dtype=np.float32)
    for p in range(ROWS):
        a, b = divmod(p % HW, 7)
        for q in range(HW):
            aq, bq = divmod(q, 7)
            if abs(aq - a) <= 3 and abs(bq - b) <= 3:
                m[p, q] = 1.0
    return m


def build():
    import ml_dtypes

    nc = bacc.Bacc(
        "TRN2", target_bir_lowering=False, debug=False, num_devices=NCORES
    )
    x_d = nc.dram_tensor("x", (B, C, H, W), F32, kind="ExternalInput")
    q_d = nc.dram_tensor("Q99", (ROWS + 1, P), F32, kind="ExternalInput")
    out_d = nc.dram_tensor("out", (B, P, H, W), F32, kind="ExternalOutput")
    # staging: inline zero constants -> pads are zero at model load and the
    # per-run dump only ever writes the data region (cols 24:122)
    stag = [
        nc.inline_tensor(
            np.zeros(SROW * ROWS, dtype=ml_dtypes.bfloat16), name=f"stag{nh}"
        )
        for nh in range(2)
    ]

    with tile.TileContext(nc) as tc:
        with (
            tc.tile_pool(name="sb", bufs=1) as sb,
            tc.tile_pool(name="ps", bufs=1, space="PSUM") as ps,
        ):
            xsb = sb.tile([128, 2, B, HW], F32)       # x, channels on partitions
            xbf = sb.tile([128, 2, B, ROWS], BF16)    # [x | ones] per (ck, n)
            gsb = sb.tile([ROWS, 2, HF], BF16)        # masked PSUM rows per half
            T = sb.tile([ROWS + 1, 2, HF], BF16)      # gathered bands + ones row
            mk = sb.tile([ROWS, HW], F32)             # band validity mask
            qq = sb.tile([ROWS + 1, P], F32)          # [Wk x49; bk x49; b_adap]
            Qbf = sb.tile([ROWS + 1, P], BF16)
            ysb = [
                sb.tile([128, 2, HF], F32, name=f"ysb{h}") for h in range(2)
            ]  # per half: (p, pk, (n2, t))

            GX_ps = ps.tile([ROWS, FREE], F32)
            Y_ps = [
                [ps.tile([128, HF], F32, name=f"y{k}_{h}") for h in range(2)]
                for k in range(2)
            ]

            mask_d = nc.inline_tensor(_mask_np(), name="mask_const")

            # x -> SBUF per (ck, half): each half completes after one DMA
            # per queue, so the first Gram starts ~0.4us earlier
            xr = x_d.ap().rearrange("n (k c) h w -> k c n (h w)", k=2)
            nc.sync.dma_start(xsb[:, 0, 0:2], xr[0][:, 0:2])
            nc.scalar.dma_start(xsb[:, 1, 0:2], xr[1][:, 0:2])
            nc.sync.dma_start(xsb[:, 0, 2:4], xr[0][:, 2:4])
            nc.scalar.dma_start(xsb[:, 1, 2:4], xr[1][:, 2:4])
            nc.sync.dma_start(mk[:], mask_d[:])
            nc.scalar.dma_start(qq[:], q_d.ap())

            # ones region of the stationary; x casts fill cols 0:49
            nc.vector.memset(xbf[:, :, :, HW:ROWS], 1.0)
            # T row 98 stays 1.0 -> b_adap via the Q ones row
            nc.vector.memset(T[:], 1.0)
            nc.gpsimd.tensor_copy(Qbf[:], qq[:])

            outr = out_d.ap().rearrange("n (k p) h w -> k p n (h w)", k=2)
            mkb = AP(mk.tensor, 0, [[HW, ROWS], [0, 2], [1, HW]])
            dumps = {}
            for nh in range(2):
                ns = slice(2 * nh, 2 * nh + 2)
                for ck in range(2):
                    nc.vector.tensor_copy(xbf[:, ck, ns, 0:HW], xsb[:, ck, ns])
                for n in range(2 * nh, 2 * nh + 2):
                    for ck in range(2):
                        nc.tensor.matmul(
                            GX_ps[:, n * HW : (n + 1) * HW],
                            xbf[:, ck, n],
                            xbf[:, ck, n, 0:HW],
                            start=(ck == 0),
                            stop=(ck == 1),
                        )
                nc.vector.tensor_tensor(
                    gsb[:, nh].rearrange("p (n t) -> p n t", n=2),
                    GX_ps[:, 2 * nh * HW : (2 * nh + 2) * HW].rearrange(
                        "p (n t) -> p n t", n=2
                    ),
                    mkb,
                    op=mybir.AluOpType.mult,
                )
                dmp = (nc.sync if nh == 0 else nc.scalar).dma_start(
                    AP(stag[nh], 24, [[SROW, ROWS], [SSEP, 2], [1, HW]]),
                    gsb[:, nh].rearrange("p (n t) -> p n t", n=2),
                )
                dumps[nh] = dmp.ins
                # one gather per region ON THE DUMP'S QUEUE with the sync
                # dep demoted to order-only: the engine issues in program
                # order and the queue ring dispatches descriptors in order
                # (196-desc distance vs ~16 in flight), so the gather's
                # reads follow the dump's writes without the ~2us
                # completion-semaphore round trip.
                for reg in range(2):
                    src = AP(
                        stag[nh],
                        SROW * HW * reg,
                        [[SROW + 1, HW], [SSEP, 2], [1, HW]],
                    )
                    dst = AP(
                        T.tensor,
                        (HW * reg) * (2 * HF) + nh * HF,
                        [[2 * HF, HW], [HW, 2], [1, HW]],
                    )
                    g = (nc.sync if nh == 0 else nc.scalar).dma_start(dst, src)
                    gi = g.ins
                    dn = dumps[nh].name
                    if dn in gi.sync_dependency_names():
                        gi.try_remove_dependency(dn)
                        add_dep_helper(
                            gi, dumps[nh], sync=False, reason="queue-ordered"
                        )
                for pk in range(2):
                    nc.tensor.matmul(
                        Y_ps[pk][nh][:],
                        Qbf[:, pk * 128 : (pk + 1) * 128],
                        T[:, nh],
                        start=True,
                        stop=True,
                    )
                    cp = nc.vector.tensor_copy if pk == 0 else nc.scalar.copy
                    cp(
                        ysb[nh][:, pk],
                        Y_ps[pk][nh][:].rearrange("p (n t) -> p n t", n=2),
                    )
                    oq = nc.sync if pk == 0 else nc.scalar
                    if nh == 0:
                        oq.dma_start(
                            outr[pk][:, ns],
                            ysb[nh][:, pk].rearrange("p (n t) -> p n t", n=2),
                        )
                    else:
                        # split the LAST outs: the 2nd piece enqueues while the
                        # queue is busy (data rides the pipeline) and halves
                        # the tail descriptor-retirement before the final wait
                        for ph in range(2):
                            pr = slice(64 * ph, 64 * (ph + 1))
                            oq.dma_start(
                                outr[pk][pr, ns],
                                ysb[nh][pr, pk].rearrange(
                                    "p (n t) -> p n t", n=2
                                ),
                            )

    nc.compile()
    return nc


def _q99(Wk, bk, b_adap):
    q = np.empty((ROWS + 1, P), dtype=np.float32)
    q[0:HW] = Wk[None, :]
    q[HW:ROWS] = bk[None, :]
    q[ROWS] = b_adap
    return q


def kernel(x, Wk, bk, b_adap):
    global last_exec_time_ns
    if "nc" not in _cached:
        _cached["nc"] = build()
    nc = _cached["nc"]

    x = np.ascontiguousarray(x, dtype=np.float32)
    q99 = np.ascontiguousarray(
        _q99(
            np.asarray(Wk, np.float32),
            np.asarray(bk, np.float32),
            np.asarray(b_adap, np.float32),
        )
    )

    in_maps = [
        {"x": x[i * B : (i + 1) * B], "Q99": q99} for i in range(NCORES)
    ]
    res = run_bass_kernel_spmd(
        nc,
        in_maps,
        core_ids=list(range(NCORES)),
        trace=bool(os.environ.get("KERNEL_TRACE")),
    )
    last_exec_time_ns = res.exec_time_ns
    out = np.concatenate(
        [res.results[i]["out"].reshape(B, P, H, W) for i in range(NCORES)], axis=0
    )
    return out

